# revision 20
# baseline (speedup 1.0000x reference)
"""Trainium2 Bass kernel for nn_ConfidenceAdaptiveSystem (MoE confidence routing).

Reference semantics (B=8192, D=4096, H=8192, C=2):
    t_out = relu(x @ t_w1 + t_b1) @ t_w2 + t_b2
    conf  = max(softmax(t_out, axis=1))          # == sigmoid(|t0 - t1|) for C=2
    f_out = relu(x @ f_w1 + f_b1) @ f_w2 + f_b2
    out   = where(conf < 0.8, f_out, t_out)

Strategy (2 launches, data-parallel over batch, 1024 rows/core):

  Main launch: the f expert runs a single-pass bf16 matmul pipeline
  (fp32 PSUM); 97% of rows are low-confidence, so f_out supplies almost
  every output value and bf16 (sigma ~1e-3 per logit) is comfortably
  inside the 2e-2 budget.  The t expert only has to supply (a) the
  routing quantity d = t0 - t1 for every row and (b) output values for
  the ~2.7% high-confidence rows, so it runs in fp8e4m3 with
  perf_mode=DoubleRow (two fp8 weights per PE cell, K=256 per pass,
  ~1.9x the bf16 matmul rate).  Inputs are scaled by SX=32 (x) and
  SW=2048 (w1) to center the fp8 dynamic range (max normal 240); the
  scale is folded back via w2/(SX*SW).  fp8 puts sigma ~0.03 on d
  (measured on seed-0 data: max |d err| 0.12), so:

  Fixup launch: rows with ||d| - ln4| < MARGIN=0.178 (6 sigma) PLUS all
  rows the fp8 d calls high-confidence (they need reference-grade t_out
  values, not just a routing bit) are gathered (measured ~423, capacity
  R_FIX=448) and their reference-grade t logits are recomputed H-sharded:
  each core does its 1024-wide H slice of matmul1 in fp32 and emits
  partial logits [2, R]; the host sums partials.  Overflow beyond R_FIX
  (or a missing fixup runner) falls back to an exact host fp64 path.

  Final select happens on host from the returned per-expert logits.
"""

import numpy as np
import ml_dtypes

import concourse.bass as bass
import concourse.mybir as mybir
from concourse.tile import TileContext
from concourse.bass_utils import run_bass_kernel_spmd

F32 = mybir.dt.float32
BF16 = mybir.dt.bfloat16
F8 = mybir.dt.float8e4
LN4 = float(np.log(0.8 / 0.2))  # conf < 0.8  <=>  |t0 - t1| < ln4
MARGIN = 0.178                  # 6 sigma of the measured fp8 d-error
R_FIX = 448                     # fixup row capacity (423 used on this data)
SX = 32.0                       # x fp8 scale (max|x|*SX ~ 173 < 240)
SW = 2048.0                     # w1 fp8 scale (max|w1|*SW ~ 173 < 240)

N_CORES = 8
B, D, H, C = 8192, 4096, 8192, 2
KT, MT = D // 128, H // 128
KP = KT // 2                    # fp8 DoubleRow k-pair count
Bc = B // N_CORES
NW = 512                        # PSUM one-bank output limit: N <= 512
NT = Bc // NW
ML = MT // N_CORES              # m-tiles per core in the H-sharded fixup


def build_main():
    """Per-core program: t expert fp8-DoubleRow, f expert bf16."""
    nc = bass.Bass(trn_type="TRN2")

    # x for the t expert: fp8, k-pair blocked: x8[kp, p, i, c] =
    # fp8(SX * x[(2kp+i)*128 + p, c])
    x8 = nc.declare_dram_parameter("x8", [KP, 128, 2, Bc], F8, isOutput=False)
    # x for the f expert: bf16, [D, Bc] transposed
    xbf = nc.declare_dram_parameter("xbf", [D, Bc], BF16, isOutput=False)
    # t weights: fp8 DoubleRow slabs: t8[m, p, kp, i, c] =
    # fp8(SW * t_w1[(2kp+i)*128 + p, m*128 + c])
    twh = nc.declare_dram_parameter("twh", [MT, 128, KP, 2, 128], F8,
                                    isOutput=False)
    # f weights: bf16 slabs: fwh[m, p, k*128+c] = bf16(f_w1[k*128+p, m*128+c])
    fwh = nc.declare_dram_parameter("fwh", [MT, 128, KT, 128], BF16,
                                    isOutput=False)
    # biases b1: [128, MT] with b1s[p, m] = b1[m*128 + p] (t's pre-scaled)
    tb1 = nc.declare_dram_parameter("tb1", [128, MT], F32, isOutput=False)
    fb1 = nc.declare_dram_parameter("fb1", [128, MT], F32, isOutput=False)
    # w2: [128, MT*2] with w2s[p, 2m:2m+2] = w2[m*128+p, :] (t's descaled)
    tw2 = nc.declare_dram_parameter("tw2", [128, MT * 2], BF16, isOutput=False)
    fw2 = nc.declare_dram_parameter("fw2", [128, MT * 2], BF16, isOutput=False)
    tlg = nc.declare_dram_parameter("tlg", [2, Bc], F32, isOutput=True)
    flg = nc.declare_dram_parameter("flg", [2, Bc], F32, isOutput=True)

    with TileContext(nc) as tc:
        with (
            tc.tile_pool(name="x8res", bufs=1) as x8pool,
            tc.tile_pool(name="xres", bufs=1) as xpool,
            tc.tile_pool(name="consts", bufs=1) as cpool,
            tc.tile_pool(name="w8stream", bufs=6) as w8pool,
            tc.tile_pool(name="wstream", bufs=7) as wpool,
            tc.tile_pool(name="hbuf", bufs=5) as hpool,
            tc.tile_pool(name="lgbuf", bufs=4) as lpool,
            tc.tile_pool(name="psmm", bufs=6, space="PSUM") as pspool,
            tc.tile_pool(name="pslg", bufs=2, space="PSUM") as ps2pool,
        ):
            # --- DMA emission order tuned for the startup window ---
            # The BSP preamble ends ~14us; the t pass gates on w8[m=0]
            # (0.5MB, split in 2 chunks) + x8[0].  The early t slabs ride
            # between x8 tiles; the f-side bulk (8MB xbf + 64MB slabs)
            # follows and streams during the ~450us t pass.
            # PE warm-up: 8 zero matmuls issue right after the BSP
            # barrier (~14us) with no DMA dependency, so the HAM clock
            # gate reaches 8/8 while the gating transfers are still in
            # flight; the early real chains are DMA-paced, so this costs
            # nothing and removes the 1.2GHz cold window (~6us).
            wz = cpool.tile([128, NW], BF16, name="wz")
            nc.vector.memset(wz[:], 0.0)
            ps_warm = pspool.tile([128, NW], F32, name="warm", tag="ps")
            for _ in range(8):
                nc.tensor.matmul(ps_warm[:], wz[:, 0:128], wz[:],
                                 start=True, stop=True)

            N_PRE = 5
            w8_pre = {}
            w8_first = w8pool.tile([128, KP, 2, 128], F8, name="w8", tag="w8")
            nc.sync.dma_start(out=w8_first[:, 0:KP // 2],
                              in_=twh[0, :, 0:KP // 2])
            x8t = []
            for kp in range(KP):
                t = x8pool.tile([128, 2, Bc], F8, name=f"x8_{kp}")
                nc.sync.dma_start(out=t[:], in_=x8[kp])
                x8t.append(t)
                if kp == 0:
                    nc.sync.dma_start(out=w8_first[:, KP // 2:],
                                      in_=twh[0, :, KP // 2:])
                if kp == 2:
                    tb1_sb = cpool.tile([128, MT], F32, name="tb1sb")
                    nc.sync.dma_start(out=tb1_sb[:], in_=tb1[:])
                    fb1_sb = cpool.tile([128, MT], F32, name="fb1sb")
                    nc.sync.dma_start(out=fb1_sb[:], in_=fb1[:])
                    tw2_sb = cpool.tile([128, MT * 2], BF16, name="tw2sb")
                    nc.sync.dma_start(out=tw2_sb[:], in_=tw2[:])
                    fw2_sb = cpool.tile([128, MT * 2], BF16, name="fw2sb")
                    nc.sync.dma_start(out=fw2_sb[:], in_=fw2[:])
                if kp >= 3 and (kp - 1) // 2 <= N_PRE:
                    m_pre = (kp - 1) // 2
                    if kp % 2 == 1:
                        wt = w8pool.tile([128, KP, 2, 128], F8, name="w8",
                                         tag="w8")
                        nc.sync.dma_start(out=wt[:], in_=twh[m_pre])
                        w8_pre[m_pre] = wt
            w8_pre[0] = w8_first
            xt = []
            for k in range(KT):
                t = xpool.tile([128, Bc], BF16, name=f"x{k}")
                nc.sync.dma_start(out=t[:], in_=xbf[k * 128:(k + 1) * 128, :])
                xt.append(t)

            for expert, (b1_sb, w2_sb, lgout) in (
                ("t", (tb1_sb, tw2_sb, tlg)),
                ("f", (fb1_sb, fw2_sb, flg)),
            ):
                # The layer-2 matmul for m is emitted after m+1's matmul1
                # chain so the PE (in-order except LDWEIGHTS pull-ahead)
                # never stalls on the activation engine producing ht.
                ps2 = [ps2pool.tile([2, NW], F32, name=f"ps2_{n}", tag="ps2")
                       for n in range(NT)]
                hts = {}

                def emit_l2(m):
                    for n in range(NT):
                        nc.tensor.matmul(
                            ps2[n][:],
                            w2_sb[:, 2 * m:2 * m + 2],
                            hts.pop((m, n))[:],
                            start=(m == 0),
                            stop=(m == MT - 1),
                        )

                for m in range(MT):
                    if expert == "t":
                        if m in w8_pre:
                            wh = w8_pre.pop(m)
                        else:
                            wh = w8pool.tile([128, KP, 2, 128], F8,
                                             name="w8", tag="w8")
                            nc.sync.dma_start(out=wh[:], in_=twh[m])
                    else:
                        wh = wpool.tile([128, KT, 128], BF16,
                                        name="wh", tag="wh")
                        nc.sync.dma_start(out=wh[:], in_=fwh[m])
                    pss = [pspool.tile([128, NW], F32, name=f"ps{n}", tag="ps")
                           for n in range(NT)]
                    if expert == "t":
                        for kp in range(KP):
                            for n in range(NT):
                                nc.tensor.matmul(
                                    pss[n][:],
                                    wh[:, kp],
                                    x8t[kp][:, :, n * NW:(n + 1) * NW],
                                    start=(kp == 0),
                                    stop=(kp == KP - 1),
                                    perf_mode=mybir.MatmulPerfMode.DoubleRow,
                                )
                    else:
                        for k in range(KT):
                            for n in range(NT):
                                nc.tensor.matmul(
                                    pss[n][:],
                                    wh[:, k],
                                    xt[k][:, n * NW:(n + 1) * NW],
                                    start=(k == 0),
                                    stop=(k == KT - 1),
                                )
                    for n in range(NT):
                        ht = hpool.tile([128, NW], BF16, name="ht", tag="ht")
                        nc.scalar.activation(
                            ht[:], pss[n][:], mybir.ActivationFunctionType.Relu,
                            bias=b1_sb[:, m:m + 1],
                        )
                        hts[(m, n)] = ht
                    if m > 0:
                        emit_l2(m - 1)
                emit_l2(MT - 1)
                for n in range(NT):
                    lg = lpool.tile([2, NW], F32, name="lg", tag="lg")
                    nc.scalar.copy(lg[:], ps2[n][:])
                    nc.sync.dma_start(
                        out=lgout[:, n * NW:(n + 1) * NW], in_=lg[:]
                    )

    _prune_weight_dma_waits(nc, {"twh", "fwh"})
    _fix_wait_overflow(nc)
    return nc


def build_fixup():
    """H-sharded reference-grade t-logit recompute for R_FIX gathered rows.

    fp32 matmuls run at quarter rate on the TRN2 PE, so the fp32-grade
    recompute is done as a bf16 hi/lo 3-pass instead (x@w ~ xh@wh +
    xh@wl + xl@wh, all accumulated in one fp32 PSUM chain; measured
    |d err| <= 1.2e-5 vs fp64, while the closest dataset row sits
    8.3e-4 from the routing threshold).  Every core gets the same x
    hi/lo [D, R_FIX] bf16 and its own 1024-wide H slice of t_w1
    (hi/lo bf16 slabs [ML, 128, KT, 128]), b1 slice [128, ML], w2
    slice [128, ML, 2] fp32.  Emits tlgp [2, R_FIX] fp32 = this
    slice's contribution to the t logits (host sums over cores and
    adds t_b2).
    """
    nc = bass.Bass(trn_type="TRN2")
    xuh = nc.declare_dram_parameter("xuh", [D, R_FIX], BF16, isOutput=False)
    xul = nc.declare_dram_parameter("xul", [D, R_FIX], BF16, isOutput=False)
    w1h = nc.declare_dram_parameter("w1h", [ML, 128, KT, 128], BF16,
                                    isOutput=False)
    w1l = nc.declare_dram_parameter("w1l", [ML, 128, KT, 128], BF16,
                                    isOutput=False)
    b1c = nc.declare_dram_parameter("b1c", [128, ML], F32, isOutput=False)
    w2c = nc.declare_dram_parameter("w2c", [128, ML, 2], F32, isOutput=False)
    tlgp = nc.declare_dram_parameter("tlgp", [2, R_FIX], F32, isOutput=True)


    with TileContext(nc) as tc:
        with (
            tc.tile_pool(name="xres", bufs=1) as xpool,
            tc.tile_pool(name="consts", bufs=1) as cpool,
            tc.tile_pool(name="wstream", bufs=12) as wpool,
            tc.tile_pool(name="hbuf", bufs=2) as hpool,
            tc.tile_pool(name="out", bufs=1) as opool,
            tc.tile_pool(name="psmm", bufs=2, space="PSUM") as pspool,
            tc.tile_pool(name="psd", bufs=1, space="PSUM") as psdpool,
        ):
            # Slab DMAs for m=1..5 interleave with the x stream so the
            # early m-chains aren't starved behind the 8MB of xh+xl
            # (each m-chain consumes 2MB of slabs per ~21us).
            wz = cpool.tile([128, R_FIX], BF16, name="wz")
            nc.vector.memset(wz[:], 0.0)
            ps_warm = pspool.tile([128, R_FIX], F32, name="warm", tag="ph")
            for _ in range(8):
                nc.tensor.matmul(ps_warm[:], wz[:, 0:128], wz[:],
                                 start=True, stop=True)

            wm_pre = {}
            w_first_h = wpool.tile([128, KT, 128], BF16, name="wm", tag="wm")
            nc.sync.dma_start(out=w_first_h[:, 0:KT // 2],
                              in_=w1h[0, :, 0:KT // 2])
            xh = []
            xl = []
            for k in range(KT):
                t = xpool.tile([128, R_FIX], BF16, name=f"xh{k}")
                nc.sync.dma_start(out=t[:], in_=xuh[k * 128:(k + 1) * 128, :])
                xh.append(t)
                if k == 0:
                    nc.sync.dma_start(out=w_first_h[:, KT // 2:],
                                      in_=w1h[0, :, KT // 2:])
                if k == 1:
                    w_first_l = wpool.tile([128, KT, 128], BF16, name="wm",
                                           tag="wm")
                    nc.sync.dma_start(out=w_first_l[:], in_=w1l[0])
                if k == 3:
                    b1_sb = cpool.tile([128, ML], F32, name="b1sb")
                    nc.sync.dma_start(out=b1_sb[:], in_=b1c[:])
                    w2_sb = cpool.tile([128, ML, 2], F32, name="w2sb")
                    nc.sync.dma_start(out=w2_sb[:], in_=w2c[:])
                if k >= 5 and k % 4 == 1 and (k - 1) // 4 <= 3:
                    m_pre = (k - 1) // 4
                    th = wpool.tile([128, KT, 128], BF16, name="wm", tag="wm")
                    nc.sync.dma_start(out=th[:], in_=w1h[m_pre])
                    tl = wpool.tile([128, KT, 128], BF16, name="wm", tag="wm")
                    nc.sync.dma_start(out=tl[:], in_=w1l[m_pre])
                    wm_pre[m_pre] = (th, tl)
            for k in range(KT):
                t = xpool.tile([128, R_FIX], BF16, name=f"xl{k}")
                nc.sync.dma_start(out=t[:], in_=xul[k * 128:(k + 1) * 128, :])
                xl.append(t)
                if k in (3, 11) and 4 + (k - 3) // 8 <= ML - 1:
                    m_pre = 4 + (k - 3) // 8
                    th = wpool.tile([128, KT, 128], BF16, name="wm", tag="wm")
                    nc.sync.dma_start(out=th[:], in_=w1h[m_pre])
                    tl = wpool.tile([128, KT, 128], BF16, name="wm", tag="wm")
                    nc.sync.dma_start(out=tl[:], in_=w1l[m_pre])
                    wm_pre[m_pre] = (th, tl)
            wm_pre[0] = (w_first_h, w_first_l)

            psd = psdpool.tile([2, R_FIX], F32, name="psd")
            for m in range(ML):
                if m in wm_pre:
                    wmh, wml = wm_pre.pop(m)
                else:
                    wmh = wpool.tile([128, KT, 128], BF16, name="wm", tag="wm")
                    nc.sync.dma_start(out=wmh[:], in_=w1h[m])
                    wml = wpool.tile([128, KT, 128], BF16, name="wm", tag="wm")
                    nc.sync.dma_start(out=wml[:], in_=w1l[m])
                ph = pspool.tile([128, R_FIX], F32, name="ph", tag="ph")
                for k in range(KT):
                    nc.tensor.matmul(ph[:], wmh[:, k], xh[k][:],
                                     start=(k == 0), stop=False)
                for k in range(KT):
                    nc.tensor.matmul(ph[:], wml[:, k], xh[k][:],
                                     start=False, stop=False)
                for k in range(KT):
                    nc.tensor.matmul(ph[:], wmh[:, k], xl[k][:],
                                     start=False, stop=(k == KT - 1))
                hu = hpool.tile([128, R_FIX], F32, name="hu", tag="hu")
                nc.scalar.activation(
                    hu[:], ph[:], mybir.ActivationFunctionType.Relu,
                    bias=b1_sb[:, m:m + 1],
                )
                nc.tensor.matmul(
                    psd[:], w2_sb[:, m], hu[:],
                    start=(m == 0), stop=(m == ML - 1),
                )
            dout = opool.tile([2, R_FIX], F32, name="dout")
            nc.scalar.copy(dout[:], psd[:])
            nc.sync.dma_start(out=tlgp[:], in_=dout[:])

    _prune_weight_dma_waits(nc, {"w1h", "w1l"})
    _fix_wait_overflow(nc)
    return nc


def _fix_wait_overflow(nc):
    """Walrus engine/DMA instructions accept at most 2 sync commands
    (waits + updates) total, but InstDrain accepts only few as well. For any
    instruction exceeding the budget, hoist the extra waits onto InstDrains
    inserted just before it on the same engine queue."""
    import concourse.mybir as _mybir

    seq = 0
    for bb in nc.m.functions[0].blocks:
        out_list = []
        for ins in bb.instructions:
            si = getattr(ins, "sync_info", None)
            if si is not None and type(ins).__name__ == "InstDrain":
                waits = list(si.on_wait or [])
                if len(waits) > 1 or len(waits) + len(si.on_update or []) > 2:
                    while len(waits) > 1:
                        chunk, waits = waits[:1], waits[1:]
                        dr = _mybir.InstDrain(
                            name=f"WOF-{seq}", engine=ins.engine, ins=[], outs=[],
                            sync_info=_mybir.SyncInfo(on_wait=chunk, on_update=[]),
                        )
                        seq += 1
                        out_list.append(dr)
                    ins.sync_info = _mybir.SyncInfo(
                        on_wait=waits, on_update=si.on_update
                    )
                out_list.append(ins)
                continue
            if (
                si is not None
                and len(si.on_wait or []) + len(si.on_update or []) > 2
            ):
                n_upd = len(si.on_update or [])
                keep = max(0, 2 - n_upd - 1) + 1 if n_upd <= 1 else 0
                keep = min(keep, len(si.on_wait))
                extras = list(si.on_wait[keep:])
                if extras:
                    for i in range(0, len(extras), 1):
                        dr = _mybir.InstDrain(
                            name=f"WOF-{seq}",
                            engine=ins.engine,
                            ins=[],
                            outs=[],
                            sync_info=_mybir.SyncInfo(
                                on_wait=extras[i:i + 1], on_update=[]
                            ),
                        )
                        seq += 1
                        out_list.append(dr)
                    ins.sync_info = _mybir.SyncInfo(
                        on_wait=list(si.on_wait[:keep]), on_update=si.on_update
                    )
            out_list.append(ins)
        bb.instructions[:] = out_list


def _prune_weight_dma_waits(nc, wsrc):
    """Walrus allows a single sem wait per DMA instruction, but Tile emits
    [engine-RAW/WAR, DMA-lane-WAW] pairs on recycled slots. The DMA-lane
    waits are redundant: the kept engine wait covers the last engine op
    touching the slot (which itself synchronized with the prior DMA), and
    same-queue DMAs execute in order regardless."""
    import concourse.mybir as _mybir

    for bb in nc.m.functions[0].blocks:
        for ins in bb.instructions:
            if type(ins).__name__ != "InstDMACopy":
                continue
            si = ins.sync_info
            if si is None or len(si.on_wait or []) <= 1:
                continue
            eng = [
                w for w in si.on_wait
                if not w.ant_name.startswith(("DMAHW", "DMASW"))
            ]
            assert len(eng) == 1, (
                f"unexpected wait mix on {ins.name}: "
                f"{[w.ant_name for w in si.on_wait]}"
            )
            src = getattr(ins.ins[0], "memref", None)
            if src in wsrc:
                assert eng[0].ant_name.startswith("PE"), eng[0].ant_name
            ins.sync_info = _mybir.SyncInfo(on_wait=eng, on_update=si.on_update)


def _prep_w1(w):
    """[D,H] -> [MT, 128, KT, 128] with w_pre[m,p,k,c] = w[k*128+p, m*128+c]"""
    return np.ascontiguousarray(
        w.reshape(KT, 128, MT, 128).transpose(2, 1, 0, 3)
    )


def _prep_w1_fp8(w):
    """[D,H] -> [MT, 128, KP, 2, 128] fp8 with
    w8[m, p, kp, i, c] = fp8(SW * w[(2kp+i)*128 + p, m*128 + c])"""
    ws = (w * SW).reshape(KP, 2, 128, MT, 128).transpose(3, 2, 0, 1, 4)
    return np.ascontiguousarray(ws).astype(ml_dtypes.float8_e4m3)


def _prep_b1(b):
    return np.ascontiguousarray(b.reshape(MT, 128).T)


def _prep_w2(w):
    return np.ascontiguousarray(
        w.reshape(MT, 128, 2).transpose(1, 0, 2).reshape(128, MT * 2)
    )


_CACHED = {}


def _get_nc(which):
    if which not in _CACHED:
        _CACHED[which] = build_main() if which == "main" else build_fixup()
    return _CACHED[which]


_RUNNER_HOOK = None  # test harness can set this to intercept executions


def _execute(nc, in_maps, label):
    if _RUNNER_HOOK is not None:
        return _RUNNER_HOOK(nc, in_maps, label)
    res = run_bass_kernel_spmd(nc, in_maps, list(range(N_CORES)), trace=False)
    return res.results


def host_prep(x, t_w1, t_b1, t_w2, t_b2, f_w1, f_b1, f_w2, f_b2):
    """All host-side packing shared by kernel() and the bench harness."""
    x = np.asarray(x, dtype=np.float32)
    t_w1 = np.asarray(t_w1, dtype=np.float32)
    f_w1 = np.asarray(f_w1, dtype=np.float32)
    t_w2 = np.asarray(t_w2, dtype=np.float32)
    f_w2 = np.asarray(f_w2, dtype=np.float32)
    t_b1 = np.asarray(t_b1, np.float32)
    f_b1 = np.asarray(f_b1, np.float32)

    tw1p = _prep_w1(t_w1)                      # fp32 slab, reused by fixup
    twh = _prep_w1_fp8(t_w1)
    fwh = _prep_w1(f_w1).astype(ml_dtypes.bfloat16)
    tw2s = _prep_w2(t_w2)
    fw2s = _prep_w2(f_w2)
    shared = dict(
        twh=twh, fwh=fwh,
        tb1=_prep_b1(t_b1 * (SX * SW)),
        fb1=_prep_b1(f_b1),
        tw2=(tw2s / (SX * SW)).astype(ml_dtypes.bfloat16),
        fw2=fw2s.astype(ml_dtypes.bfloat16),
    )
    main_maps = []
    for c in range(N_CORES):
        xc = np.ascontiguousarray(x[c * Bc:(c + 1) * Bc].T)   # [D, Bc]
        x8c = np.ascontiguousarray(
            (xc * SX).reshape(KP, 2, 128, Bc).transpose(0, 2, 1, 3)
        ).astype(ml_dtypes.float8_e4m3)                       # [KP,128,2,Bc]
        main_maps.append(dict(
            shared, xbf=xc.astype(ml_dtypes.bfloat16), x8=x8c
        ))

    # fixup constants per core (H slice), xu hi/lo filled in later
    tb1s = _prep_b1(t_b1)
    w2rs = t_w2.reshape(MT, 128, 2)            # [m, p, c]
    fix_shared = []
    for c in range(N_CORES):
        w2c = np.ascontiguousarray(
            w2rs[c * ML:(c + 1) * ML].transpose(1, 0, 2)      # [128, ML, 2]
        )
        wsl = tw1p[c * ML:(c + 1) * ML]
        w1h = wsl.astype(ml_dtypes.bfloat16)
        w1l = (wsl - w1h.astype(np.float32)).astype(ml_dtypes.bfloat16)
        fix_shared.append(dict(
            w1h=np.ascontiguousarray(w1h),
            w1l=np.ascontiguousarray(w1l),
            b1c=np.ascontiguousarray(tb1s[:, c * ML:(c + 1) * ML]),
            w2c=w2c,
        ))

    global _W1_REF, _B1_REF, _W2_REF
    _W1_REF = t_w1
    _B1_REF = t_b1
    _W2_REF = t_w2
    return x, main_maps, fix_shared


def finish(x, res_main, fix_shared, t_b2, f_b2, run_fixup=None):
    """Host routing + select.

    fp8 t logits carry sigma~0.03 on d; rows inside the MARGIN band or
    called high-confidence get reference-grade t logits from the fixup
    launch (device, H-sharded fp32).  Overflow beyond R_FIX and the
    no-runner case use an exact host fp64 recompute instead.
    """
    t_b2 = np.asarray(t_b2, np.float32)
    f_b2 = np.asarray(f_b2, np.float32)
    tl = np.concatenate([res_main[c]["tlg"] for c in range(N_CORES)], axis=1)
    fl = np.concatenate([res_main[c]["flg"] for c in range(N_CORES)], axis=1)
    t_out = tl.T + t_b2[None, :]               # [B, 2]
    f_out = fl.T + f_b2[None, :]
    d = t_out[:, 0] - t_out[:, 1]

    need = np.nonzero(
        (np.abs(np.abs(d) - LN4) < MARGIN) | (np.abs(d) >= LN4)
    )[0]
    if len(need) > 0 and run_fixup is not None:
        rows = need[:R_FIX]
        xu = np.zeros((D, R_FIX), np.float32)
        xu[:, :len(rows)] = x[rows].T
        xuh = xu.astype(ml_dtypes.bfloat16)
        xul = (xu - xuh.astype(np.float32)).astype(ml_dtypes.bfloat16)
        fix_maps = [dict(fs, xuh=xuh, xul=xul) for fs in fix_shared]
        res_fix = run_fixup(fix_maps)
        lg = np.zeros((2, R_FIX), np.float64)
        for c in range(N_CORES):
            lg += res_fix[c]["tlgp"].astype(np.float64)
        t_exact = lg.T[:len(rows)] + t_b2[None, :].astype(np.float64)
        t_out[rows] = t_exact.astype(np.float32)
        d[rows] = (t_exact[:, 0] - t_exact[:, 1]).astype(np.float32)
        need = need[R_FIX:]                    # host path handles overflow
    if len(need) > 0:
        h = np.maximum(
            x[need].astype(np.float64) @ _W1_REF.astype(np.float64)
            + _B1_REF.astype(np.float64)[None, :],
            0.0,
        )
        t_exact = h @ _W2_REF.astype(np.float64) + t_b2.astype(np.float64)
        t_out[need] = t_exact.astype(np.float32)
        d[need] = (t_exact[:, 0] - t_exact[:, 1]).astype(np.float32)
    low_conf = np.abs(d) < LN4
    out = np.where(low_conf[:, None], f_out, t_out)
    return np.ascontiguousarray(out.astype(np.float32))


_W1_REF = None
_B1_REF = None
_W2_REF = None


def kernel(x, t_w1, t_b1, t_w2, t_b2, f_w1, f_b1, f_w2, f_b2):
    x, main_maps, fix_shared = host_prep(
        x, t_w1, t_b1, t_w2, t_b2, f_w1, f_b1, f_w2, f_b2
    )
    res_main = _execute(_get_nc("main"), main_maps, "main")

    def run_fixup(fix_maps):
        return _execute(_get_nc("fixup"), fix_maps, "fixup")

    return finish(x, res_main, fix_shared, t_b2, f_b2, run_fixup=run_fixup)
